# revision 10
# baseline (speedup 1.0000x reference)
"""HGAT retrieval-kNN kernel for Trainium2, data-parallel over batch on 8 cores.

Pipeline per batch element (reference semantics):
  pre = W @ x + b                               [128, 1024]
  pairwise = -||pre_v - pre_u||^2 per vertex    [1024, 1024]
  idx = top_k(pairwise, 32) indices             [1024, 32]
  s[v,k] = q[(32v+k) % 1024] + r[idx[v,k]],  q = a1.T pre, r = a2.T pre
  H = softmax(s, axis=batch)

Rank-equivalent distance (per row v, constants in v dropped):
  z[v,u] = x_v . (M x)_u + (c . x_u - 0.5*||pre_u||^2),
  M = W^T W [64,64], c = W^T b [64].
The column-only term rides in an augmented 65th contraction row
(lhsT row 64 = ones, rhs row 64 = c.x - 0.5*xx), so one K=65 fp32
matmul per 512-col half produces z directly in PSUM -- half the PE
work of the K=128 pre-Gram plus broadcast.

Top-32 per row: exact DVE max8/max_index rounds; the match_replace
step is rank-safe offloaded when OFFLOAD_REPLACE=True.
Host: gather r by idx, add q, softmax over batch.
"""

import numpy as np

B, C_IN, V = 32, 64, 1024
C_REL, K = 128, 32
N_CORES = 8
BPC = B // N_CORES  # 4 batches per core
NEG = -3.0e38

OFFLOAD_REPLACE = True

_cache = {}


def _build():
    import concourse.bacc as bacc
    import concourse.mybir as mybir
    import concourse.tile as tile

    dt = mybir.dt
    AF = mybir.ActivationFunctionType
    AO = mybir.AluOpType
    nc = bacc.Bacc(None, target_bir_lowering=False, debug=False)

    x_d = nc.dram_tensor("x", [BPC, C_IN, V], dt.float32, kind="ExternalInput")
    w_d = nc.dram_tensor("w", [C_REL, C_IN], dt.float32, kind="ExternalInput")
    wt_d = nc.dram_tensor("wt", [C_IN, C_REL], dt.float32, kind="ExternalInput")
    bias_d = nc.dram_tensor("bias", [C_REL, 1], dt.float32, kind="ExternalInput")
    a12_d = nc.dram_tensor("a12", [C_REL, 2], dt.float32, kind="ExternalInput")
    mi_d = nc.dram_tensor("mi", [BPC, 128, 256], dt.uint16, kind="ExternalOutput")
    qr_d = nc.dram_tensor("qr", [BPC, 2, V], dt.float32, kind="ExternalOutput")

    with tile.TileContext(nc) as tc:
        with tc.tile_pool(name="const", bufs=1) as cpool, \
             tc.tile_pool(name="perb", bufs=2) as bpool, \
             tc.tile_pool(name="zsb", bufs=4) as zpool, \
             tc.tile_pool(name="mvp", bufs=4) as mvpool, \
             tc.tile_pool(name="sgp", bufs=2) as sgpool, \
             tc.tile_pool(name="psz", bufs=2, space="PSUM") as psz, \
             tc.tile_pool(name="psp", bufs=2, space="PSUM") as psp, \
             tc.tile_pool(name="pss", bufs=2, space="PSUM") as pss:

            w_sb = cpool.tile([C_REL, C_IN], dt.float32)
            nc.sync.dma_start(w_sb[:], w_d[:])
            wt_sb = cpool.tile([C_IN, C_REL], dt.float32)
            nc.sync.dma_start(wt_sb[:], wt_d[:])
            bias_sb = cpool.tile([C_REL, 1], dt.float32)
            nc.sync.dma_start(bias_sb[:], bias_d[:])
            a12_sb = cpool.tile([C_REL, 2], dt.float32)
            nc.sync.dma_start(a12_sb[:], a12_d[:])
            ones_c = cpool.tile([C_REL, 1], dt.float32)
            nc.vector.memset(ones_c[:], 1.0)

            # m65T [64, 65]: cols 0-63 = M = W^T W, col 64 = c = W^T b.
            # (matmul computes lhsT.T @ rhs with contraction on partitions.)
            # PSUM scratch borrows a zp-pool slot to stay within 8 banks.
            m65T = cpool.tile([C_IN, 65], dt.float32)
            pm = psz.tile([128, 1024], dt.float32, tag="zp")
            nc.tensor.matmul(pm[0:C_IN, 0:64], w_sb[:], w_sb[:, 0:64],
                             start=True, stop=True)
            nc.tensor.matmul(pm[0:C_IN, 64:65], w_sb[:], bias_sb[:],
                             start=True, stop=True)
            nc.scalar.copy(m65T[:], pm[0:C_IN, 0:65])

            for b in range(BPC):
                # x lands in rows 0-63 of the augmented rhs tile; row 64
                # gets c.x - 0.5*xx once nxx and mx are ready.
                xr = bpool.tile([65, V], dt.float32, tag="xr")
                nc.sync.dma_start(xr[0:C_IN, 0:512], x_d[b][:, 0:512])
                nc.sync.dma_start(xr[0:C_IN, 512:1024], x_d[b][:, 512:1024])

                # pre = W @ x + bias; xx = sum_c pre^2; nxx = -0.5*xx
                pre_sb = bpool.tile([C_REL, V], dt.float32, tag="pre")
                pre2 = bpool.tile([C_REL, V], dt.float32, tag="pre2")
                nxx_sb = bpool.tile([1, V], dt.float32, tag="nxx")
                mxl = bpool.tile([65, V], dt.float32, tag="mxl")
                pmx = psz.tile([128, 1024], dt.float32, tag="zp")
                for h in range(2):
                    hs = slice(h * 512, (h + 1) * 512)
                    pp = psp.tile([C_REL, 512], dt.float32, tag="pp")
                    nc.tensor.matmul(pp[:], wt_sb[:], xr[0:C_IN, hs],
                                     start=True, stop=True)
                    nc.scalar.activation(pre_sb[:, hs], pp[:],
                                         AF.Identity, bias=bias_sb[:], scale=1.0)
                    nc.scalar.square(pre2[:, hs], pre_sb[:, hs])
                    pxx = pss.tile([2, 512], dt.float32, tag="pxs")
                    nc.tensor.matmul(pxx[0:1, :], ones_c[:], pre2[:, hs],
                                     start=True, stop=True)
                    nc.scalar.activation(nxx_sb[:, hs], pxx[0:1, :],
                                         AF.Copy, scale=-0.5)

                    # mx = [M; c^T] @ x : rows 0-63 -> mxl, row 64 = c.x
                    nc.tensor.matmul(pmx[0:65, hs], m65T[:], xr[0:C_IN, hs],
                                     start=True, stop=True)
                    nc.scalar.copy(mxl[0:C_IN, hs], pmx[0:C_IN, hs])
                    # xr row 64 = c.x + nxx   (PSUM + SBUF add on DVE, tiny)
                    nc.vector.tensor_tensor(out=xr[64:65, hs],
                                            in0=nxx_sb[:, hs],
                                            in1=pmx[64:65, hs],
                                            op=AO.add)
                # mxl row 64 = 1.0 (ACT: Copy(nxx*0 + 1))
                nc.scalar.activation(mxl[64:65, :], nxx_sb[:],
                                     AF.Copy, bias=1.0, scale=0.0)

                mi_sb = bpool.tile([128, 256], dt.uint16, tag="mi")
                # Chunks processed in pairs with their top-k rounds
                # interleaved, so the gpsimd replace of one chunk hides
                # behind the sibling chunk's DVE ops (engine queues are
                # in-order; without interleave each round stalls DVE).
                for cp in range(4):
                    zs, mvs, sgs = [], [], []
                    for e in range(2):
                        c = cp * 2 + e
                        zp = psz.tile([128, 1024], dt.float32, tag="zp")
                        for h in range(2):
                            hs = slice(h * 512, (h + 1) * 512)
                            nc.tensor.matmul(zp[:, hs],
                                             mxl[:, c * 128:(c + 1) * 128],
                                             xr[:, hs],
                                             start=True, stop=True)
                        z_sb = zpool.tile([128, V], dt.float32, tag="z",
                                          name=f"z_{cp}_{e}")
                        nc.scalar.copy(z_sb[:], zp[:])
                        mv_sb = mvpool.tile([128, 32], dt.float32, tag="mv",
                                            name=f"mv_{cp}_{e}")
                        sg_sb = (sgpool.tile([128, V], dt.float32, tag="sg",
                                             name=f"sg_{cp}_{e}")
                                 if OFFLOAD_REPLACE else None)
                        zs.append(z_sb)
                        mvs.append(mv_sb)
                        sgs.append(sg_sb)
                    for rnd in range(4):
                        rs = slice(rnd * 8, (rnd + 1) * 8)
                        for e in range(2):
                            c = cp * 2 + e
                            z_sb, mv_sb, sg_sb = zs[e], mvs[e], sgs[e]
                            o = c * 32 + rnd * 8
                            nc.vector.max(out=mv_sb[:, rs], in_=z_sb[:])
                            nc.vector.max_index(out=mi_sb[:, o:o + 8],
                                                in_max=mv_sb[:, rs], in_values=z_sb[:])
                            if rnd < 3:
                                if OFFLOAD_REPLACE:
                                    t_ap = mv_sb[:, rnd * 8 + 7:rnd * 8 + 8]
                                    # mb = (z >= t) * -BIG  (DVE tensor_scalar,
                                    # single-src fp32 SBUF -> 2x_2P mode)
                                    nc.vector.tensor_scalar(
                                        out=sg_sb[:], in0=z_sb[:],
                                        scalar1=t_ap, scalar2=-1.0e30,
                                        op0=AO.is_ge, op1=AO.mult)
                                    # z += mb  (gpsimd; -0.0 add keeps survivors exact)
                                    nc.gpsimd.tensor_tensor(
                                        out=z_sb[:], in0=z_sb[:], in1=sg_sb[:],
                                        op=AO.add)
                                else:
                                    nc.vector.match_replace(out=z_sb[:], in_to_replace=mv_sb[:, rs],
                                                            in_values=z_sb[:], imm_value=NEG)
                nc.sync.dma_start(mi_d[b], mi_sb[:])

                # q, r rows off the critical path
                qr_sb = bpool.tile([2, V], dt.float32, tag="qr")
                for h in range(2):
                    pqr = pss.tile([2, 512], dt.float32, tag="pxs")
                    nc.tensor.matmul(pqr[:], a12_sb[:],
                                     pre_sb[:, h * 512:(h + 1) * 512],
                                     start=True, stop=True)
                    nc.scalar.copy(qr_sb[:, h * 512:(h + 1) * 512], pqr[:])
                nc.sync.dma_start(qr_d[b], qr_sb[:])

    nc.compile()
    return nc


def _get_nc():
    if "nc" not in _cache:
        _cache["nc"] = _build()
    return _cache["nc"]


def kernel(x, W, b_conv, a):
    from concourse import bass_utils

    x = np.ascontiguousarray(np.asarray(x, dtype=np.float32))
    W = np.asarray(W, dtype=np.float32)
    b_conv = np.asarray(b_conv, dtype=np.float32)
    a = np.asarray(a, dtype=np.float32)

    nc = _get_nc()

    w = np.ascontiguousarray(W)                         # [128, 64]
    wt = np.ascontiguousarray(W.T)                      # [64, 128]
    bias = np.ascontiguousarray(b_conv[:, None])        # [128, 1]
    a12 = np.ascontiguousarray(
        np.stack([a[:C_REL, 0], a[C_REL:, 0]], axis=1)  # [128, 2]
    )
    xs = x.reshape(N_CORES, BPC, C_IN, V)

    in_maps = [{"x": np.ascontiguousarray(xs[c]), "w": w, "wt": wt,
                "bias": bias, "a12": a12}
               for c in range(N_CORES)]
    res = bass_utils.run_bass_kernel_spmd(nc, in_maps, list(range(N_CORES)))

    # host finish: gather r, add q, softmax over batch
    idx = np.empty((B, V, K), dtype=np.int64)
    q = np.empty((B, V), dtype=np.float32)
    r = np.empty((B, V), dtype=np.float32)
    for c in range(N_CORES):
        out = res.results[c]
        mi = out["mi"].reshape(BPC, 128, 8, K).transpose(0, 2, 1, 3).reshape(BPC, V, K)
        idx[c * BPC:(c + 1) * BPC] = mi
        q[c * BPC:(c + 1) * BPC] = out["qr"][:, 0, :]
        r[c * BPC:(c + 1) * BPC] = out["qr"][:, 1, :]

    pos = (np.arange(V)[:, None] * K + np.arange(K)[None, :]) % V    # [V, K]
    s = q[:, pos] + np.take_along_axis(r, idx.reshape(B, V * K), axis=1).reshape(B, V, K)
    s = s.astype(np.float32)
    m = s.max(axis=0, keepdims=True)
    e = np.exp(s - m, dtype=np.float32)
    H = e / e.sum(axis=0, keepdims=True)
    return H.astype(np.float32)


# revision 13
# speedup vs baseline: 1.2803x; 1.2803x over previous
"""HGAT retrieval-kNN kernel for Trainium2, data-parallel over batch on 8 cores.

Reference semantics per batch element:
  pre = W @ x + b; pairwise = -||pre_v - pre_u||^2; idx = top_k(pairwise, 32)
  s[v,k] = q[(32v+k) % 1024] + r[idx[v,k]] with q = a1.pre, r = a2.pre
  H = softmax(s, axis=batch)

Device-side reduction (rank/softmax-equivalent):
  With M = W^T W, every bias term either is constant per top-k row or
  cancels:  z*[v,u] = x_v.(Mx)_u - 0.5 * x_u.(Mx)_u
  ranks identically to pairwise per row, and q' = (W^T a1).x,
  r' = (W^T a2).x differ from q, r by batch-independent constants that
  cancel in the softmax over the batch axis.  So the device needs only:
  Mx (K=64 matmul), xmx = x*Mx elementwise, one K=65 augmented matmul
  per 512-col half for z* (lhsT row 64 = ones, rhs row 64 =
  -0.5*colsum(xmx)), and the exact DVE top-32 (max8 / max_index /
  match_replace rounds, tie-break identical to jax.lax.top_k).
Host: gather r' by idx, add q', softmax over batch.
"""

import numpy as np

B, C_IN, V = 32, 64, 1024
C_REL, K = 128, 32
N_CORES = 8
BPC = B // N_CORES  # 4 batches per core
NEG = -3.0e38

_cache = {}


def _build():
    import concourse.bacc as bacc
    import concourse.mybir as mybir
    import concourse.tile as tile

    dt = mybir.dt
    AF = mybir.ActivationFunctionType
    AO = mybir.AluOpType
    nc = bacc.Bacc(None, target_bir_lowering=False, debug=False)

    x_d = nc.dram_tensor("x", [BPC, C_IN, V], dt.float32, kind="ExternalInput")
    w_d = nc.dram_tensor("w", [C_REL, C_IN], dt.float32, kind="ExternalInput")
    a12_d = nc.dram_tensor("a12", [C_REL, 2], dt.float32, kind="ExternalInput")
    mi_d = nc.dram_tensor("mi", [BPC, 128, 256], dt.uint16, kind="ExternalOutput")
    qr_d = nc.dram_tensor("qr", [BPC, 2, V], dt.float32, kind="ExternalOutput")

    with tile.TileContext(nc) as tc:
        with tc.tile_pool(name="const", bufs=1) as cpool, \
             tc.tile_pool(name="perb", bufs=2) as bpool, \
             tc.tile_pool(name="zsb", bufs=6) as zpool, \
             tc.tile_pool(name="mvp", bufs=8) as mvpool, \
             tc.tile_pool(name="sgp", bufs=4) as sgpool, \
             tc.tile_pool(name="pszh", bufs=4, space="PSUM") as pszh, \
             tc.tile_pool(name="psm", bufs=2, space="PSUM") as psm, \
             tc.tile_pool(name="pss", bufs=2, space="PSUM") as pss:

            w_sb = cpool.tile([C_REL, C_IN], dt.float32)
            nc.sync.dma_start(w_sb[:], w_d[:])
            a12_sb = cpool.tile([C_REL, 2], dt.float32)
            nc.sync.dma_start(a12_sb[:], a12_d[:])
            mhalf = cpool.tile([C_IN, 1], dt.float32)
            nc.vector.memset(mhalf[:], -0.5)
            bigc = cpool.tile([128, 1], dt.float32)
            nc.vector.memset(bigc[:], 1.0e30)
            nbigc = cpool.tile([128, 1], dt.float32)
            nc.vector.memset(nbigc[:], -1.0e30)
            halfc = cpool.tile([128, 1], dt.float32)
            nc.vector.memset(halfc[:], 0.5)

            # M = W^T W [64,64]; wa = W^T [a1 a2] [64,2]  (one-time)
            m_sb = cpool.tile([C_IN, C_IN], dt.float32)
            wa_sb = cpool.tile([C_IN, 2], dt.float32)
            pm = psm.tile([65, 512], dt.float32, tag="pmx")
            nc.tensor.matmul(pm[0:C_IN, 0:C_IN], w_sb[:], w_sb[:, 0:C_IN],
                             start=True, stop=True)
            nc.scalar.copy(m_sb[:], pm[0:C_IN, 0:C_IN])
            pwa = psm.tile([65, 512], dt.float32, tag="pmx")
            nc.tensor.matmul(pwa[0:C_IN, 0:2], w_sb[:], a12_sb[:],
                             start=True, stop=True)
            nc.scalar.copy(wa_sb[:], pwa[0:C_IN, 0:2])

            for b in range(BPC):
                # xr: rows 0-63 = x, row 64 = -0.5*colsum(x*Mx)
                # mxl: rows 0-63 = Mx, row 64 = ones
                xr = bpool.tile([65, V], dt.float32, tag="xr")
                nc.sync.dma_start(xr[0:C_IN, 0:512], x_d[b][:, 0:512])
                nc.sync.dma_start(xr[0:C_IN, 512:1024], x_d[b][:, 512:1024])

                mxl = bpool.tile([65, V], dt.float32, tag="mxl")
                xmx = bpool.tile([C_IN, V], dt.float32, tag="xmx")
                qr_sb = bpool.tile([2, V], dt.float32, tag="qr")
                for h in range(2):
                    hs = slice(h * 512, (h + 1) * 512)
                    pmx = psm.tile([65, 512], dt.float32, tag="pmx")
                    nc.tensor.matmul(pmx[0:C_IN, :], m_sb[:], xr[0:C_IN, hs],
                                     start=True, stop=True)
                    nc.scalar.copy(mxl[0:C_IN, hs], pmx[0:C_IN, :])
                    # xmx = x * Mx (gpsimd, off the hot engines)
                    nc.gpsimd.tensor_tensor(out=xmx[:, hs], in0=xr[0:C_IN, hs],
                                            in1=mxl[0:C_IN, hs], op=AO.mult)
                    prow = pss.tile([2, 512], dt.float32, tag="pxs")
                    nc.tensor.matmul(prow[0:1, :], mhalf[:], xmx[:, hs],
                                     start=True, stop=True)
                    nc.scalar.copy(xr[64:65, hs], prow[0:1, :])
                    # q', r'
                    pqr = pss.tile([2, 512], dt.float32, tag="pxs")
                    nc.tensor.matmul(pqr[:], wa_sb[:], xr[0:C_IN, hs],
                                     start=True, stop=True)
                    nc.scalar.copy(qr_sb[:, hs], pqr[:])
                # mxl row 64 = 1.0 (Copy(in*0 + 1); input is arbitrary)
                nc.scalar.activation(mxl[64:65, :], xmx[0:1, :],
                                     AF.Copy, bias=1.0, scale=0.0)
                nc.sync.dma_start(qr_d[b], qr_sb[:])

                mi_sb = bpool.tile([128, 256], dt.uint16, tag="mi")
                for c in range(8):
                    # z*[v,u] = sum_c Mx[c,v] x[c,u] + row64_u   (K=65)
                    z_sb = zpool.tile([128, V], dt.float32, tag="z")
                    for h in range(2):
                        hs = slice(h * 512, (h + 1) * 512)
                        zp = pszh.tile([128, 512], dt.float32, tag="zph")
                        nc.tensor.matmul(zp[:],
                                         mxl[:, c * 128:(c + 1) * 128],
                                         xr[:, hs],
                                         start=True, stop=True)
                        nc.scalar.copy(z_sb[:, hs], zp[:])

                    # exact top-32 (values discarded, indices kept).
                    # The replace step runs off-DVE: ACT computes
                    # m=Sign(t-z) then Prelu(BIG*m-BIG, a=0.5) which maps
                    # {+1,0,-1} -> {0,-BIG/2,-BIG} (exactly 0 for z<t),
                    # and gpsimd adds it into z.  Survivor bits unchanged.
                    mv_sb = mvpool.tile([128, 32], dt.float32, tag="mv")
                    sg_sb = sgpool.tile([128, V], dt.float32, tag="sg")
                    for rnd in range(4):
                        rs = slice(rnd * 8, (rnd + 1) * 8)
                        o = c * 32 + rnd * 8
                        nc.vector.max(out=mv_sb[:, rs], in_=z_sb[:])
                        nc.vector.max_index(out=mi_sb[:, o:o + 8],
                                            in_max=mv_sb[:, rs], in_values=z_sb[:])
                        if rnd < 3:
                            t_ap = mv_sb[:, rnd * 8 + 7:rnd * 8 + 8]
                            nc.scalar.activation(sg_sb[:], z_sb[:],
                                                 AF.Sign, bias=t_ap, scale=-1.0)
                            nc.scalar.activation(sg_sb[:], sg_sb[:],
                                                 AF.Prelu, bias=nbigc[:],
                                                 scale=bigc[:], alpha=halfc[:])
                            nc.gpsimd.tensor_tensor(out=z_sb[:], in0=z_sb[:],
                                                    in1=sg_sb[:], op=AO.add)
                nc.sync.dma_start(mi_d[b], mi_sb[:])

    nc.compile()
    return nc


def _get_nc():
    if "nc" not in _cache:
        _cache["nc"] = _build()
    return _cache["nc"]


def kernel(x, W, b_conv, a):
    from concourse import bass_utils

    x = np.ascontiguousarray(np.asarray(x, dtype=np.float32))
    W = np.asarray(W, dtype=np.float32)
    a = np.asarray(a, dtype=np.float32)

    nc = _get_nc()

    w = np.ascontiguousarray(W)                         # [128, 64]
    a12 = np.ascontiguousarray(
        np.stack([a[:C_REL, 0], a[C_REL:, 0]], axis=1)  # [128, 2]
    )
    xs = x.reshape(N_CORES, BPC, C_IN, V)

    in_maps = [{"x": np.ascontiguousarray(xs[c]), "w": w, "a12": a12}
               for c in range(N_CORES)]
    res = bass_utils.run_bass_kernel_spmd(nc, in_maps, list(range(N_CORES)))

    # host finish: gather r', add q', softmax over batch (constant offsets
    # q-q' and r-r' are batch-independent and cancel in the softmax)
    idx = np.empty((B, V, K), dtype=np.int64)
    q = np.empty((B, V), dtype=np.float32)
    r = np.empty((B, V), dtype=np.float32)
    for c in range(N_CORES):
        out = res.results[c]
        mi = out["mi"].reshape(BPC, 128, 8, K).transpose(0, 2, 1, 3).reshape(BPC, V, K)
        idx[c * BPC:(c + 1) * BPC] = mi
        q[c * BPC:(c + 1) * BPC] = out["qr"][:, 0, :]
        r[c * BPC:(c + 1) * BPC] = out["qr"][:, 1, :]

    pos = (np.arange(V)[:, None] * K + np.arange(K)[None, :]) % V    # [V, K]
    s = q[:, pos] + np.take_along_axis(r, idx.reshape(B, V * K), axis=1).reshape(B, V, K)
    s = s.astype(np.float32)
    m = s.max(axis=0, keepdims=True)
    e = np.exp(s - m, dtype=np.float32)
    H = e / e.sum(axis=0, keepdims=True)
    return H.astype(np.float32)


# revision 14
# speedup vs baseline: 1.3216x; 1.0322x over previous
"""HGAT retrieval-kNN kernel for Trainium2, data-parallel over batch on 8 cores.

Reference semantics per batch element:
  pre = W @ x + b; pairwise = -||pre_v - pre_u||^2; idx = top_k(pairwise, 32)
  s[v,k] = q[(32v+k) % 1024] + r[idx[v,k]] with q = a1.pre, r = a2.pre
  H = softmax(s, axis=batch)

Device-side reduction (rank/softmax-equivalent):
  With M = W^T W, every bias term either is constant per top-k row or
  cancels:  z*[v,u] = x_v.(Mx)_u - 0.5 * x_u.(Mx)_u
  ranks identically to pairwise per row, and q' = (W^T a1).x,
  r' = (W^T a2).x differ from q, r by batch-independent constants that
  cancel in the softmax over the batch axis.  So the device needs only:
  Mx (K=64 matmul), xmx = x*Mx elementwise, one K=65 augmented matmul
  per 512-col half for z* (lhsT row 64 = ones, rhs row 64 =
  -0.5*colsum(xmx)), and the exact DVE top-32 (max8 / max_index /
  match_replace rounds, tie-break identical to jax.lax.top_k).
Host: gather r' by idx, add q', softmax over batch.
"""

import numpy as np

B, C_IN, V = 32, 64, 1024
C_REL, K = 128, 32
N_CORES = 8
BPC = B // N_CORES  # 4 batches per core
NEG = -3.0e38

_cache = {}


def _build():
    import concourse.bacc as bacc
    import concourse.mybir as mybir
    import concourse.tile as tile

    dt = mybir.dt
    AF = mybir.ActivationFunctionType
    AO = mybir.AluOpType
    nc = bacc.Bacc(None, target_bir_lowering=False, debug=False)

    x_d = nc.dram_tensor("x", [BPC, C_IN, V], dt.float32, kind="ExternalInput")
    w_d = nc.dram_tensor("w", [C_REL, C_IN], dt.float32, kind="ExternalInput")
    a12_d = nc.dram_tensor("a12", [C_REL, 2], dt.float32, kind="ExternalInput")
    mi_d = nc.dram_tensor("mi", [BPC, 128, 256], dt.uint16, kind="ExternalOutput")
    qr_d = nc.dram_tensor("qr", [BPC, 2, V], dt.float32, kind="ExternalOutput")

    with tile.TileContext(nc) as tc:
        with tc.tile_pool(name="const", bufs=1) as cpool, \
             tc.tile_pool(name="perb", bufs=2) as bpool, \
             tc.tile_pool(name="zsb", bufs=6) as zpool, \
             tc.tile_pool(name="mvp", bufs=8) as mvpool, \
             tc.tile_pool(name="sgp", bufs=4) as sgpool, \
             tc.tile_pool(name="pszh", bufs=4, space="PSUM") as pszh, \
             tc.tile_pool(name="psm", bufs=2, space="PSUM") as psm, \
             tc.tile_pool(name="pss", bufs=2, space="PSUM") as pss:

            w_sb = cpool.tile([C_REL, C_IN], dt.float32)
            nc.sync.dma_start(w_sb[:], w_d[:])
            a12_sb = cpool.tile([C_REL, 2], dt.float32)
            nc.sync.dma_start(a12_sb[:], a12_d[:])
            mhalf = cpool.tile([C_IN, 1], dt.float32)
            nc.vector.memset(mhalf[:], -0.5)
            bigc = cpool.tile([128, 1], dt.float32)
            nc.vector.memset(bigc[:], 1.0e30)
            nbigc = cpool.tile([128, 1], dt.float32)
            nc.vector.memset(nbigc[:], -1.0e30)
            halfc = cpool.tile([128, 1], dt.float32)
            nc.vector.memset(halfc[:], 0.5)

            # M = W^T W [64,64]; wa = W^T [a1 a2] [64,2]  (one-time)
            m_sb = cpool.tile([C_IN, C_IN], dt.float32)
            wa_sb = cpool.tile([C_IN, 2], dt.float32)
            pm = psm.tile([65, 512], dt.float32, tag="pmx")
            nc.tensor.matmul(pm[0:C_IN, 0:C_IN], w_sb[:], w_sb[:, 0:C_IN],
                             start=True, stop=True)
            nc.scalar.copy(m_sb[:], pm[0:C_IN, 0:C_IN])
            pwa = psm.tile([65, 512], dt.float32, tag="pmx")
            nc.tensor.matmul(pwa[0:C_IN, 0:2], w_sb[:], a12_sb[:],
                             start=True, stop=True)
            nc.scalar.copy(wa_sb[:], pwa[0:C_IN, 0:2])

            for b in range(BPC):
                # xr: rows 0-63 = x, row 64 = -0.5*colsum(x*Mx)
                # mxl: rows 0-63 = Mx, row 64 = ones
                xr = bpool.tile([65, V], dt.float32, tag="xr")
                nc.sync.dma_start(xr[0:C_IN, 0:512], x_d[b][:, 0:512])
                nc.sync.dma_start(xr[0:C_IN, 512:1024], x_d[b][:, 512:1024])

                mxl = bpool.tile([65, V], dt.float32, tag="mxl")
                xmx = bpool.tile([C_IN, V], dt.float32, tag="xmx")
                qr_sb = bpool.tile([2, V], dt.float32, tag="qr")
                for h in range(2):
                    hs = slice(h * 512, (h + 1) * 512)
                    pmx = psm.tile([65, 512], dt.float32, tag="pmx")
                    nc.tensor.matmul(pmx[0:C_IN, :], m_sb[:], xr[0:C_IN, hs],
                                     start=True, stop=True)
                    nc.scalar.copy(mxl[0:C_IN, hs], pmx[0:C_IN, :])
                    # xmx = x * Mx (gpsimd, off the hot engines)
                    nc.gpsimd.tensor_tensor(out=xmx[:, hs], in0=xr[0:C_IN, hs],
                                            in1=mxl[0:C_IN, hs], op=AO.mult)
                    prow = pss.tile([2, 512], dt.float32, tag="pxs")
                    nc.tensor.matmul(prow[0:1, :], mhalf[:], xmx[:, hs],
                                     start=True, stop=True)
                    nc.scalar.copy(xr[64:65, hs], prow[0:1, :])
                    # q', r'
                    pqr = pss.tile([2, 512], dt.float32, tag="pxs")
                    nc.tensor.matmul(pqr[:], wa_sb[:], xr[0:C_IN, hs],
                                     start=True, stop=True)
                    nc.scalar.copy(qr_sb[:, hs], pqr[:])
                # mxl row 64 = 1.0 (Copy(in*0 + 1); input is arbitrary)
                nc.scalar.activation(mxl[64:65, :], xmx[0:1, :],
                                     AF.Copy, bias=1.0, scale=0.0)
                nc.sync.dma_start(qr_d[b], qr_sb[:])

                mi_sb = bpool.tile([128, 256], dt.uint16, tag="mi")
                for c in range(8):
                    # z*[v,u] = sum_c Mx[c,v] x[c,u] + row64_u   (K=65)
                    z_sb = zpool.tile([128, V], dt.float32, tag="z")
                    for h in range(2):
                        hs = slice(h * 512, (h + 1) * 512)
                        zp = pszh.tile([128, 512], dt.float32, tag="zph")
                        nc.tensor.matmul(zp[:],
                                         mxl[:, c * 128:(c + 1) * 128],
                                         xr[:, hs],
                                         start=True, stop=True)
                        nc.scalar.copy(z_sb[:, hs], zp[:])

                    # exact top-32 (values discarded, indices kept).
                    # The replace step runs off-DVE: ACT computes
                    # m=Sign(t-z) then Prelu(BIG*m-BIG, a=0.5) which maps
                    # {+1,0,-1} -> {0,-BIG/2,-BIG} (exactly 0 for z<t),
                    # and gpsimd adds it into z.  Survivor bits unchanged.
                    mv_sb = mvpool.tile([128, 32], dt.float32, tag="mv")
                    sg_sb = sgpool.tile([128, V], dt.float32, tag="sg")
                    for rnd in range(4):
                        rs = slice(rnd * 8, (rnd + 1) * 8)
                        o = c * 32 + rnd * 8
                        nc.vector.max(out=mv_sb[:, rs], in_=z_sb[:])
                        nc.vector.max_index(out=mi_sb[:, o:o + 8],
                                            in_max=mv_sb[:, rs], in_values=z_sb[:])
                        if rnd < 3:
                            t_ap = mv_sb[:, rnd * 8 + 7:rnd * 8 + 8]
                            # halves pipeline the Sign->Prelu->add chain
                            for hh in range(2):
                                h2 = slice(hh * 512, (hh + 1) * 512)
                                nc.scalar.activation(sg_sb[:, h2], z_sb[:, h2],
                                                     AF.Sign, bias=t_ap, scale=-1.0)
                                nc.scalar.activation(sg_sb[:, h2], sg_sb[:, h2],
                                                     AF.Prelu, bias=nbigc[:],
                                                     scale=bigc[:], alpha=halfc[:])
                                nc.gpsimd.tensor_tensor(out=z_sb[:, h2],
                                                        in0=z_sb[:, h2],
                                                        in1=sg_sb[:, h2], op=AO.add)
                nc.sync.dma_start(mi_d[b], mi_sb[:])

    nc.compile()
    return nc


def _get_nc():
    if "nc" not in _cache:
        _cache["nc"] = _build()
    return _cache["nc"]


def kernel(x, W, b_conv, a):
    from concourse import bass_utils

    x = np.ascontiguousarray(np.asarray(x, dtype=np.float32))
    W = np.asarray(W, dtype=np.float32)
    a = np.asarray(a, dtype=np.float32)

    nc = _get_nc()

    w = np.ascontiguousarray(W)                         # [128, 64]
    a12 = np.ascontiguousarray(
        np.stack([a[:C_REL, 0], a[C_REL:, 0]], axis=1)  # [128, 2]
    )
    xs = x.reshape(N_CORES, BPC, C_IN, V)

    in_maps = [{"x": np.ascontiguousarray(xs[c]), "w": w, "a12": a12}
               for c in range(N_CORES)]
    res = bass_utils.run_bass_kernel_spmd(nc, in_maps, list(range(N_CORES)))

    # host finish: gather r', add q', softmax over batch (constant offsets
    # q-q' and r-r' are batch-independent and cancel in the softmax)
    idx = np.empty((B, V, K), dtype=np.int64)
    q = np.empty((B, V), dtype=np.float32)
    r = np.empty((B, V), dtype=np.float32)
    for c in range(N_CORES):
        out = res.results[c]
        mi = out["mi"].reshape(BPC, 128, 8, K).transpose(0, 2, 1, 3).reshape(BPC, V, K)
        idx[c * BPC:(c + 1) * BPC] = mi
        q[c * BPC:(c + 1) * BPC] = out["qr"][:, 0, :]
        r[c * BPC:(c + 1) * BPC] = out["qr"][:, 1, :]

    pos = (np.arange(V)[:, None] * K + np.arange(K)[None, :]) % V    # [V, K]
    s = q[:, pos] + np.take_along_axis(r, idx.reshape(B, V * K), axis=1).reshape(B, V, K)
    s = s.astype(np.float32)
    m = s.max(axis=0, keepdims=True)
    e = np.exp(s - m, dtype=np.float32)
    H = e / e.sum(axis=0, keepdims=True)
    return H.astype(np.float32)
